# revision 8
# baseline (speedup 1.0000x reference)
"""Trainium2 Bass kernel for nn_AbstractRelu (DeepPoly abstract-ReLU transform).

The reference's piecewise-linear transform reduces exactly to:
    x_out    = relu(x)
    high_out = relu(high)        (crossing branch: w_high*high + b_high == high)
    low_out  = low if low + high >= 0 else 0
and `relu(high)` can replace `high` in the low_out test without changing any
result (when high <= 0, low < high <= 0 forces low + high < 0 AND low < 0).

The problem is pure HBM bandwidth (elementwise, 6 streams). The 2e-2 rel-err
budget admits bf16 for everything except the branch decision, so:
  - x is pre-cast to bf16 on the host (relu preserves sign, so only the
    bf16 rounding of the value itself shows up: rel err <= 2^-9),
  - low/high are read in f32 (the mask low+high>=0 must match the f32
    reference bit-exactly -- a flipped boundary element is rel err 1.0),
  - all three outputs are written as bf16 and upcast to f32 on the host.
Per-core traffic: 4 MiB (x) + 16 MiB (low,high) reads + 12 MiB writes
= 33.5 MiB vs 48 MiB all-f32.

Sharding: N=16.7M elements split evenly across 8 NeuronCores; fully
elementwise, no communication.
"""

import numpy as np

import concourse.bass as bass
import concourse.bacc as bacc
import concourse.mybir as mybir
from concourse.tile import TileContext
from concourse.bass_utils import run_bass_kernel_spmd

N = 16777216
N_CORES = 8
SHARD = N // N_CORES  # 2_097_152
P = 128
FREE = SHARD // P  # 16384 elements per partition per core
TILE_COLS = 4096
N_TILES = FREE // TILE_COLS
F32 = mybir.dt.float32
BF16 = mybir.dt.bfloat16
NP_BF16 = mybir.dt.np(BF16)


def build_program(
    free: int = FREE,
    tile_cols: int = TILE_COLS,
    bufs: int = 3,
    repeats: int = 1,
    hw_loop_repeats: int = 1,
    store_engine: str = "gpsimd",
    load_engine: str = "split",
    x_relu_on_dve: bool = False,
    compute: bool = True,
    schedule: str = "v1",
) -> bass.Bass:
    """hw_loop_repeats wraps the whole body in a tc.For_i hardware loop --
    used only by the timing harness (repeat-differencing)."""
    assert free % tile_cols == 0
    n_tiles = free // tile_cols

    nc = bacc.Bacc(
        "TRN2", target_bir_lowering=False, debug=False, num_devices=N_CORES
    )
    # Each DRAM tile [P, tile_cols] is one fully contiguous block in HBM
    # (best row-buffer locality); the host reshapes to match.
    shape = [n_tiles, P, tile_cols]
    x = nc.declare_dram_parameter("x", shape, BF16, isOutput=False)
    low = nc.declare_dram_parameter("low", shape, F32, isOutput=False)
    high = nc.declare_dram_parameter("high", shape, F32, isOutput=False)
    x_out = nc.declare_dram_parameter("x_out", shape, BF16, isOutput=True)
    low_out = nc.declare_dram_parameter("low_out", shape, BF16, isOutput=True)
    high_out = nc.declare_dram_parameter("high_out", shape, BF16, isOutput=True)

    relu = mybir.ActivationFunctionType.Relu
    with TileContext(nc) as tc:
        with tc.tile_pool(name="io", bufs=bufs) as pool:
            engines = {"scalar": nc.scalar, "gpsimd": nc.gpsimd, "sync": nc.sync}

            def eng_for(stream: str, t: int):
                """Resolve the DMA-issuing engine for stream in
                {x,h,l,xo,ho,lo} at tile t. Loads stay on the two HWDGE
                rings (sync/scalar) so they are never head-of-line blocked
                behind stores, which wait on compute; stores go to SWDGE
                (gpsimd) by default."""
                if stream in ("x", "h", "l"):
                    if load_engine == "split":
                        # balance HWDGE ring bytes: h(f32) on scalar,
                        # l(f32) on sync, x(bf16, half-size) alternates
                        if stream == "h":
                            return engines["scalar"]
                        if stream == "l":
                            return engines["sync"]
                        return engines["sync" if t % 2 == 0 else "scalar"]
                    return engines[load_engine]
                if store_engine == "mix":
                    return engines["scalar" if stream == "xo" else "gpsimd"]
                if store_engine == "alt":
                    return engines["gpsimd" if t % 2 == 0 else "scalar"]
                return engines[store_engine]

            def body():
                for t in range(n_tiles * repeats):
                    ti = t % n_tiles

                    xt = pool.tile([P, tile_cols], BF16, tag="x")
                    eng_for("x", t).dma_start(out=xt[:], in_=x[ti])
                    if compute:
                        if x_relu_on_dve:
                            nc.vector.tensor_scalar_max(xt[:], xt[:], 0.0)
                        else:
                            nc.scalar.activation(xt[:], xt[:], relu)
                    eng_for("xo", t).dma_start(out=x_out[ti], in_=xt[:])

                    ht = pool.tile([P, tile_cols], F32, tag="h")
                    eng_for("h", t).dma_start(out=ht[:], in_=high[ti])
                    lt = pool.tile([P, tile_cols], F32, tag="l")
                    eng_for("l", t).dma_start(out=lt[:], in_=low[ti])

                    if not compute:
                        # DMA-floor diagnostic: identical transfer shapes,
                        # no compute ops (stores the loaded bytes as-is)
                        eng_for("ho", t).dma_start(
                            out=high_out[ti],
                            in_=ht[:].bitcast(BF16)[:, 0:tile_cols],
                        )
                        eng_for("lo", t).dma_start(
                            out=low_out[ti],
                            in_=lt[:].bitcast(BF16)[:, 0:tile_cols],
                        )
                        continue

                    ho = pool.tile([P, tile_cols], BF16, tag="ho")
                    nc.scalar.activation(ho[:], ht[:], relu)  # f32 -> bf16
                    eng_for("ho", t).dma_start(out=high_out[ti], in_=ho[:])

                    # s = low + high computed in place over ht (f32, exact);
                    # mask = (s >= 0); low_out = mask * low, rounded to bf16
                    nc.vector.tensor_add(ht[:], lt[:], ht[:])
                    nc.vector.tensor_scalar(
                        ht[:], ht[:], 0.0, None, mybir.AluOpType.is_ge
                    )
                    lo = pool.tile([P, tile_cols], BF16, tag="lo")
                    nc.vector.tensor_mul(lo[:], ht[:], lt[:])
                    eng_for("lo", t).dma_start(out=low_out[ti], in_=lo[:])

            def body_v2():
                """Per-engine streams with no compute op ever blocking a
                load DMA issue:
                  sync   : h,l loads only (pure DMA, runs ahead of compute)
                  scalar : self-paced x pipeline (load -> relu -> store;
                           every dependency is satisfied engine-locally)
                  vector : relu(high)->bf16 and the whole low chain
                  gpsimd : ho,lo stores (inherently compute-dependent)
                """
                for t in range(n_tiles * repeats):
                    ti = t % n_tiles

                    ht = pool.tile([P, tile_cols], F32, tag="h")
                    nc.sync.dma_start(out=ht[:], in_=high[ti])
                    lt = pool.tile([P, tile_cols], F32, tag="l")
                    nc.sync.dma_start(out=lt[:], in_=low[ti])

                    xt = pool.tile([P, tile_cols], BF16, tag="x")
                    nc.scalar.dma_start(out=xt[:], in_=x[ti])
                    nc.scalar.activation(xt[:], xt[:], relu)
                    nc.scalar.dma_start(out=x_out[ti], in_=xt[:])

                    ho = pool.tile([P, tile_cols], BF16, tag="ho")
                    nc.vector.tensor_scalar_max(ho[:], ht[:], 0.0)  # f32->bf16
                    nc.gpsimd.dma_start(out=high_out[ti], in_=ho[:])

                    nc.vector.tensor_add(ht[:], lt[:], ht[:])
                    nc.vector.tensor_scalar(
                        ht[:], ht[:], 0.0, None, mybir.AluOpType.is_ge
                    )
                    lo = pool.tile([P, tile_cols], BF16, tag="lo")
                    nc.vector.tensor_mul(lo[:], ht[:], lt[:])
                    nc.gpsimd.dma_start(out=low_out[ti], in_=lo[:])

            def body_v3():
                """Both HWDGE rings are pure load streams; all compute on
                DVE; all stores on SWDGE."""
                for t in range(n_tiles * repeats):
                    ti = t % n_tiles

                    ht = pool.tile([P, tile_cols], F32, tag="h")
                    nc.scalar.dma_start(out=ht[:], in_=high[ti])
                    lt = pool.tile([P, tile_cols], F32, tag="l")
                    nc.sync.dma_start(out=lt[:], in_=low[ti])
                    xt = pool.tile([P, tile_cols], BF16, tag="x")
                    (nc.sync if t % 2 == 0 else nc.scalar).dma_start(
                        out=xt[:], in_=x[ti]
                    )

                    nc.vector.tensor_scalar_max(xt[:], xt[:], 0.0)
                    nc.gpsimd.dma_start(out=x_out[ti], in_=xt[:])

                    ho = pool.tile([P, tile_cols], BF16, tag="ho")
                    nc.vector.tensor_scalar_max(ho[:], ht[:], 0.0)  # f32->bf16
                    nc.gpsimd.dma_start(out=high_out[ti], in_=ho[:])

                    nc.vector.tensor_add(ht[:], lt[:], ht[:])
                    nc.vector.tensor_scalar(
                        ht[:], ht[:], 0.0, None, mybir.AluOpType.is_ge
                    )
                    lo = pool.tile([P, tile_cols], BF16, tag="lo")
                    nc.vector.tensor_mul(lo[:], ht[:], lt[:])
                    nc.gpsimd.dma_start(out=low_out[ti], in_=lo[:])

            def body_v4(prefetch: int = 2, ho_on_dve: bool = False):
                """Software-prefetched loads: tile t+PF's loads are issued
                before tile t's compute in every engine's program order, so
                a compute op on scalar never delays a load issue by more
                than the PF-tile slack. Loads: h->scalar, l->sync, x
                alternating; relus on scalar (DVE keeps only the 3-op low
                chain); stores on gpsimd."""
                total = n_tiles * repeats

                def issue_loads(t):
                    ti = t % n_tiles
                    ht = pool.tile([P, tile_cols], F32, tag="h")
                    nc.scalar.dma_start(out=ht[:], in_=high[ti])
                    lt = pool.tile([P, tile_cols], F32, tag="l")
                    nc.sync.dma_start(out=lt[:], in_=low[ti])
                    xt = pool.tile([P, tile_cols], BF16, tag="x")
                    (nc.sync if t % 2 == 0 else nc.scalar).dma_start(
                        out=xt[:], in_=x[ti]
                    )
                    return xt, ht, lt

                from collections import deque

                q = deque()
                for t in range(min(prefetch, total)):
                    q.append(issue_loads(t))
                for t in range(total):
                    if t + prefetch < total:
                        q.append(issue_loads(t + prefetch))
                    ti = t % n_tiles
                    xt, ht, lt = q.popleft()

                    nc.scalar.activation(xt[:], xt[:], relu)
                    nc.gpsimd.dma_start(out=x_out[ti], in_=xt[:])

                    ho = pool.tile([P, tile_cols], BF16, tag="ho")
                    if ho_on_dve:
                        nc.vector.tensor_scalar_max(ho[:], ht[:], 0.0)
                    else:
                        nc.scalar.activation(ho[:], ht[:], relu)
                    nc.gpsimd.dma_start(out=high_out[ti], in_=ho[:])

                    nc.vector.tensor_add(ht[:], lt[:], ht[:])
                    nc.vector.tensor_scalar(
                        ht[:], ht[:], 0.0, None, mybir.AluOpType.is_ge
                    )
                    lo = pool.tile([P, tile_cols], BF16, tag="lo")
                    nc.vector.tensor_mul(lo[:], ht[:], lt[:])
                    nc.gpsimd.dma_start(out=low_out[ti], in_=lo[:])

            def body_v5():
                """v3 for low/high, but the x stream never touches SBUF:
                x_out is zero-filled once (prologue), then each tile does a
                DRAM->DRAM max-accumulate DMA (SWDGE CCE): x_out =
                max(x_out, x) = relu(x). Idempotent across hw-loop
                repeats. Saves 8.4 MB/core of SBUF-AXI traffic per
                iteration at the cost of extra HBM-side traffic."""
                for t in range(n_tiles * repeats):
                    ti = t % n_tiles

                    nc.gpsimd.dma_start(
                        out=x_out[ti], in_=x[ti], accum_op=mybir.AluOpType.max
                    )

                    ht = pool.tile([P, tile_cols], F32, tag="h")
                    nc.scalar.dma_start(out=ht[:], in_=high[ti])
                    lt = pool.tile([P, tile_cols], F32, tag="l")
                    nc.sync.dma_start(out=lt[:], in_=low[ti])

                    ho = pool.tile([P, tile_cols], BF16, tag="ho")
                    nc.vector.tensor_scalar_max(ho[:], ht[:], 0.0)  # f32->bf16
                    nc.gpsimd.dma_start(out=high_out[ti], in_=ho[:])

                    nc.vector.tensor_add(ht[:], lt[:], ht[:])
                    nc.vector.tensor_scalar(
                        ht[:], ht[:], 0.0, None, mybir.AluOpType.is_ge
                    )
                    lo = pool.tile([P, tile_cols], BF16, tag="lo")
                    nc.vector.tensor_mul(lo[:], ht[:], lt[:])
                    nc.gpsimd.dma_start(out=low_out[ti], in_=lo[:])

            def prologue_v5():
                # zero x_out so the first max-accum pass computes relu;
                # issued on the same SWDGE ring as the accum DMAs, which
                # execute FIFO per ring slice, so ordering is guaranteed.
                zt = pool.tile([P, tile_cols], BF16, tag="z")
                nc.vector.memset(zt[:], 0.0)
                for ti in range(n_tiles):
                    nc.gpsimd.dma_start(out=x_out[ti], in_=zt[:])

            body_fn = {
                "v1": body,
                "v2": body_v2,
                "v3": body_v3,
                "v4": body_v4,
                "v4d": lambda: body_v4(ho_on_dve=True),
                "v4p3": lambda: body_v4(prefetch=3),
                "v4p4": lambda: body_v4(prefetch=4),
                "v5": body_v5,
            }[schedule]
            if schedule == "v5":
                prologue_v5()
            if hw_loop_repeats > 1:
                with tc.For_i(0, hw_loop_repeats, 1):
                    body_fn()
            else:
                body_fn()
    nc.compile()
    return nc


def shard_inputs(x, low, high, tile_cols: int = TILE_COLS):
    """Full f32 arrays -> per-core input dicts matching the BIR declaration
    (x cast to bf16; contiguous-tile layout [n_tiles, P, tile_cols])."""
    x = np.ascontiguousarray(np.asarray(x, dtype=np.float32).reshape(-1))
    low = np.ascontiguousarray(np.asarray(low, dtype=np.float32).reshape(-1))
    high = np.ascontiguousarray(np.asarray(high, dtype=np.float32).reshape(-1))
    assert x.shape == (N,), x.shape
    xb = x.astype(NP_BF16)  # round-to-nearest-even
    shard_shape = (FREE // tile_cols, P, tile_cols)
    in_maps = []
    for c in range(N_CORES):
        s = slice(c * SHARD, (c + 1) * SHARD)
        in_maps.append(
            {
                "x": xb[s].reshape(shard_shape),
                "low": low[s].reshape(shard_shape),
                "high": high[s].reshape(shard_shape),
            }
        )
    return in_maps


_NC = None


def _get_nc() -> bass.Bass:
    global _NC
    if _NC is None:
        _NC = build_program()
    return _NC


_RUNNER = None


def _make_runner(nc):
    """Cached PJRT runner (mirrors bass2jax.run_bass_via_pjrt, but the jitted
    callable is built once so repeat kernel() calls skip re-tracing). No
    donation: this kernel writes every output element, so the zero 'output'
    operands are reusable dummies and XLA result buffers may start uninit."""
    import jax
    from jax.sharding import Mesh, PartitionSpec, NamedSharding
    from jax.experimental.shard_map import shard_map
    from concourse.bass2jax import (
        _bass_exec_p,
        install_neuronx_cc_hook,
        partition_id_tensor,
    )

    install_neuronx_cc_hook()
    partition_name = nc.partition_id_tensor.name if nc.partition_id_tensor else None

    in_names, out_names, out_avals, zero_shapes = [], [], [], []
    for alloc in nc.m.functions[0].allocations:
        if not isinstance(alloc, mybir.MemoryLocationSet):
            continue
        name = alloc.memorylocations[0].name
        if alloc.kind == "ExternalInput":
            if name != partition_name:
                in_names.append(name)
        elif alloc.kind == "ExternalOutput":
            shape = tuple(alloc.tensor_shape)
            dtype = mybir.dt.np(alloc.dtype)
            out_names.append(name)
            out_avals.append(jax.core.ShapedArray(shape, dtype))
            zero_shapes.append((shape, dtype))
    n_params = len(in_names)
    all_in_names = list(in_names) + list(out_names)
    if partition_name is not None:
        all_in_names.append(partition_name)

    def _body(*args):
        operands = list(args)
        if partition_name is not None:
            operands.append(partition_id_tensor())
        outs = _bass_exec_p.bind(
            *operands,
            out_avals=tuple(out_avals),
            in_names=tuple(all_in_names),
            out_names=tuple(out_names),
            lowering_input_output_aliases=(),
            sim_require_finite=True,
            sim_require_nnan=True,
            nc=nc,
        )
        return tuple(outs)

    devices = jax.devices()[:N_CORES]
    mesh = Mesh(np.asarray(devices), ("core",))
    n_io = n_params + len(out_names)
    sharded = jax.jit(
        shard_map(
            _body,
            mesh=mesh,
            in_specs=(PartitionSpec("core"),) * n_io,
            out_specs=(PartitionSpec("core"),) * len(out_names),
            check_rep=False,
        ),
        keep_unused=True,
    )
    sharding = NamedSharding(mesh, PartitionSpec("core"))
    zeros = [
        jax.device_put(np.zeros((N_CORES * s[0], *s[1:]), d), sharding)
        for (s, d) in zero_shapes
    ]

    def run(in_maps):
        concat_in = [
            np.concatenate([np.asarray(in_maps[c][nm]) for c in range(N_CORES)], axis=0)
            for nm in in_names
        ]
        dev_in = [jax.device_put(a, sharding) for a in concat_in]
        outs = sharded(*dev_in, *zeros)
        return {
            nm: np.asarray(outs[i]).reshape(N_CORES, *out_avals[i].shape)
            for i, nm in enumerate(out_names)
        }

    return run


def kernel(x: np.ndarray, low: np.ndarray, high: np.ndarray, **_run_kwargs):
    nc = _get_nc()
    in_maps = shard_inputs(x, low, high)
    global _RUNNER
    results = None
    if not _run_kwargs:
        # Fast path: cached jitted executable (no per-call re-trace).
        try:
            if _RUNNER is None:
                _RUNNER = _make_runner(nc)
            by_name = _RUNNER(in_maps)
            results = [
                {nm: by_name[nm][c] for nm in by_name} for c in range(N_CORES)
            ]
        except Exception:
            _RUNNER = None
            results = None

    if results is None:
        res = None
        for attempt in range(3):
            try:
                res = run_bass_kernel_spmd(
                    nc, in_maps, list(range(N_CORES)), **_run_kwargs
                )
                break
            except Exception:
                # Transient device wedge (NRT_EXEC_UNIT_UNRECOVERABLE) -- reset
                # the jax backend so the next attempt re-establishes the mesh.
                if attempt == 2:
                    raise
                import time as _time

                try:
                    import jax

                    jax.clear_caches()
                    jax.extend.backend.clear_backends()
                except Exception:
                    pass
                _time.sleep(10.0)
        results = res.results
        if _run_kwargs:
            kernel.last_results = res  # expose trace/profile to test harness

    x_out = np.concatenate([results[c]["x_out"].reshape(-1) for c in range(N_CORES)])
    low_out = np.concatenate([results[c]["low_out"].reshape(-1) for c in range(N_CORES)])
    high_out = np.concatenate([results[c]["high_out"].reshape(-1) for c in range(N_CORES)])
    return (
        x_out.astype(np.float32),
        low_out.astype(np.float32),
        high_out.astype(np.float32),
    )
